# revision 4
# baseline (speedup 1.0000x reference)
# MoE (8 experts, top-2, SwiGLU) Trainium2 kernel.
#
# Strategy (expert-parallel, as suggested by the sharding hint):
#   - Host: compute router (logits -> top-2 -> softmax weights), build the
#     per-expert token lists (the "all-to-all dispatch"), gather + pad the
#     tokens for each expert, and pre-transpose/pad the weight stacks.
#   - Device (8 cores, SPMD, core e owns expert e): dense SwiGLU FFN over the
#     gathered tokens in bf16 with fp32 PSUM accumulation; each token's
#     output is scaled by its routing weight on-device.
#   - Host: scatter-add the two per-expert contributions back to [N, C].
#
# Shapes (hardcoded for this problem):
#   x [2, 2048, 1024] f32, gate_w [8, 1024], w1/w2 [8, 2730, 1024],
#   w3 [8, 1024, 2730].  N = 4096 tokens, C = 1024, H = 2730 (padded 2816).

import numpy as np
import ml_dtypes

NUM_EXPERTS = 8
TOP_K = 2
C = 1024
H = 2730
H2 = 2816  # H padded to a multiple of 128 (zero rows contribute nothing)
KC = C // 128  # 8 contraction chunks over C
MH = H2 // 128  # 22 chunks over padded H
N_CORES = 8

_bf16 = ml_dtypes.bfloat16

_program_cache: dict[int, object] = {}


def _route_host(xt: np.ndarray, gate_w: np.ndarray):
    """Mirror of the reference router in fp32 numpy.

    logits = xt @ gate_w.T; top-2 (ties -> lower index, like jax top_k);
    softmax over the two selected logits.
    """
    logits = xt @ gate_w.T.astype(np.float32)  # [N, E] fp32
    # top-1: first argmax; top-2: argmax with top-1 masked out
    i1 = np.argmax(logits, axis=1)
    n_idx = np.arange(logits.shape[0])
    v1 = logits[n_idx, i1]
    masked = logits.copy()
    masked[n_idx, i1] = -np.inf
    i2 = np.argmax(masked, axis=1)
    v2 = masked[n_idx, i2]
    # softmax over [v1, v2] in fp32 (v1 >= v2)
    e2 = np.exp((v2 - v1).astype(np.float32))
    w1 = (1.0 / (1.0 + e2)).astype(np.float32)
    w2 = (e2 / (1.0 + e2)).astype(np.float32)
    top_idx = np.stack([i1, i2], axis=1)  # [N, 2]
    top_w = np.stack([w1, w2], axis=1)  # [N, 2] fp32
    return top_idx, top_w


def _token_tiles(cap: int):
    tiles = []
    n0 = 0
    while n0 < cap:
        nw = min(512, cap - n0)
        tiles.append((n0, nw))
        n0 += nw
    return tiles


def _build_program(cap: int):
    """Build the SPMD Bass program for per-core token capacity `cap`."""
    import concourse.bass as bass
    import concourse.mybir as mybir
    from concourse import bacc
    from concourse.tile import TileContext

    dt = mybir.dt
    TN = cap // 128
    tiles = _token_tiles(cap)

    nc = bacc.Bacc(None, target_bir_lowering=False)
    xgT_d = nc.declare_dram_parameter("xgT", [KC, 128, cap], dt.bfloat16, isOutput=False)
    w1T_d = nc.declare_dram_parameter("w1T", [MH, 128, KC, 128], dt.bfloat16, isOutput=False)
    w2T_d = nc.declare_dram_parameter("w2T", [MH, 128, KC, 128], dt.bfloat16, isOutput=False)
    w3T_d = nc.declare_dram_parameter("w3T", [MH, 128, C], dt.bfloat16, isOutput=False)
    wtT_d = nc.declare_dram_parameter("wtT", [128, TN], dt.float32, isOutput=False)
    out_d = nc.declare_dram_parameter("out", [TN, 128, C], dt.float32, isOutput=True)

    with TileContext(nc) as tc:
        with (
            tc.tile_pool(name="big", bufs=1) as big,
            tc.tile_pool(name="wstream", bufs=3) as wpool,
            tc.tile_pool(name="work", bufs=3) as work,
            tc.tile_pool(name="psum", bufs=2, space="PSUM") as psum,
        ):
            # Resident SBUF tensors
            xg_sb = big.tile([128, KC, cap], dt.bfloat16)
            act_sb = big.tile([128, MH, cap], dt.bfloat16)
            w3_sb = big.tile([128, MH, C], dt.bfloat16)
            wt_sb = big.tile([128, TN], dt.float32)

            nc.sync.dma_start(out=wt_sb[:], in_=wtT_d[:])
            for k in range(KC):
                nc.sync.dma_start(out=xg_sb[:, k, :], in_=xgT_d[k])

            # ---- Phase 1: h1/h2 matmuls + SwiGLU -> act_sb ----
            for m in range(MH):
                w1s = wpool.tile([128, KC, 128], dt.bfloat16, tag="w1s")
                w2s = wpool.tile([128, KC, 128], dt.bfloat16, tag="w2s")
                nc.sync.dma_start(out=w1s[:], in_=w1T_d[m])
                nc.sync.dma_start(out=w2s[:], in_=w2T_d[m])
                # w3 chunk for phase 2, loaded here to spread DMA traffic
                nc.sync.dma_start(out=w3_sb[:, m, :], in_=w3T_d[m])

                for (n0, nw) in tiles:
                    ps1 = psum.tile([128, nw], dt.float32, tag="ps1", padded_shape=[128, 512])
                    ps2 = psum.tile([128, nw], dt.float32, tag="ps2", padded_shape=[128, 512])
                    for k in range(KC):
                        nc.tensor.matmul(
                            ps1[:], lhsT=w1s[:, k, :], rhs=xg_sb[:, k, n0:n0 + nw],
                            start=(k == 0), stop=(k == KC - 1),
                        )
                    for k in range(KC):
                        nc.tensor.matmul(
                            ps2[:], lhsT=w2s[:, k, :], rhs=xg_sb[:, k, n0:n0 + nw],
                            start=(k == 0), stop=(k == KC - 1),
                        )
                    tmp = work.tile([128, nw], dt.float32, tag="tmp", padded_shape=[128, 512])
                    nc.scalar.activation(tmp[:], ps1[:], mybir.ActivationFunctionType.Silu)
                    nc.vector.tensor_mul(act_sb[:, m, n0:n0 + nw], tmp[:], ps2[:])

            # ---- Phase 2: out = (act @ w3T) * wt ----
            for tn in range(TN):
                ps3 = [
                    psum.tile([128, 512], dt.float32, tag=f"ps3_{co}",
                              name=f"ps3_{co}_{tn}")
                    for co in range(2)
                ]
                for m in range(MH):
                    for co in range(2):
                        nc.tensor.matmul(
                            ps3[co][:],
                            lhsT=act_sb[:, m, tn * 128:(tn + 1) * 128],
                            rhs=w3_sb[:, m, co * 512:(co + 1) * 512],
                            start=(m == 0), stop=(m == MH - 1),
                        )
                for co in range(2):
                    o_sb = work.tile([128, 512], dt.float32, tag="osb", bufs=4)
                    nc.vector.tensor_scalar_mul(o_sb[:], ps3[co][:], wt_sb[:, tn:tn + 1])
                    nc.sync.dma_start(
                        out=out_d[tn][:, co * 512:(co + 1) * 512], in_=o_sb[:]
                    )

    nc.finalize()  # runs bacc legalization (e.g. multi-wait split for TRN2)
    return nc


def _prepare_core_inputs(xt, w1, w2, w3, top_idx, top_w):
    """Host-side dispatch: gather tokens per expert, pad, transpose, cast."""
    N = xt.shape[0]
    idx_lists = []
    wt_lists = []
    for e in range(NUM_EXPERTS):
        m0 = top_idx[:, 0] == e
        m1 = top_idx[:, 1] == e
        sel = m0 | m1
        idx_e = np.nonzero(sel)[0]
        wt_e = np.where(m0[idx_e], top_w[idx_e, 0], top_w[idx_e, 1]).astype(np.float32)
        idx_lists.append(idx_e)
        wt_lists.append(wt_e)

    max_cnt = max(len(i) for i in idx_lists)
    cap = max(128, ((max_cnt + 127) // 128) * 128)
    TN = cap // 128

    in_maps = []
    for e in range(NUM_EXPERTS):
        idx_e = idx_lists[e]
        cnt = len(idx_e)
        xg = np.zeros((cap, C), np.float32)
        xg[:cnt] = xt[idx_e]
        xgT = np.ascontiguousarray(xg.T.reshape(KC, 128, cap)).astype(_bf16)

        w1p = np.zeros((H2, C), np.float32)
        w1p[:H] = w1[e]
        w2p = np.zeros((H2, C), np.float32)
        w2p[:H] = w2[e]
        w3p = np.zeros((C, H2), np.float32)
        w3p[:, :H] = w3[e]

        # [MH, 128(part=c within chunk), KC, 128(h within chunk)]
        w1T = np.ascontiguousarray(
            w1p.T.reshape(KC, 128, MH, 128).transpose(2, 1, 0, 3)
        ).astype(_bf16)
        w2T = np.ascontiguousarray(
            w2p.T.reshape(KC, 128, MH, 128).transpose(2, 1, 0, 3)
        ).astype(_bf16)
        # [MH, 128(part=h within chunk), C]
        w3T = np.ascontiguousarray(w3p.T.reshape(MH, 128, C)).astype(_bf16)

        wt_pad = np.zeros(cap, np.float32)
        wt_pad[:cnt] = wt_lists[e]
        wtT = np.ascontiguousarray(wt_pad.reshape(TN, 128).T)

        in_maps.append({
            "xgT": xgT, "w1T": w1T, "w2T": w2T, "w3T": w3T, "wtT": wtT,
        })
    return in_maps, idx_lists, cap


def _run(x, gate_w, w1, w2, w3, trace=False):
    from concourse.bass_utils import run_bass_kernel_spmd

    x = np.asarray(x, dtype=np.float32)
    gate_w = np.asarray(gate_w, dtype=np.float32)
    w1 = np.asarray(w1, dtype=np.float32)
    w2 = np.asarray(w2, dtype=np.float32)
    w3 = np.asarray(w3, dtype=np.float32)

    B, T, Cx = x.shape
    assert Cx == C
    xt = x.reshape(-1, C)
    N = xt.shape[0]

    top_idx, top_w = _route_host(xt, gate_w)
    in_maps, idx_lists, cap = _prepare_core_inputs(xt, w1, w2, w3, top_idx, top_w)

    if cap not in _program_cache:
        _program_cache[cap] = _build_program(cap)
    nc = _program_cache[cap]

    res = run_bass_kernel_spmd(nc, in_maps, list(range(N_CORES)), trace=trace)

    out = np.zeros((N, C), np.float32)
    for e in range(NUM_EXPERTS):
        idx_e = idx_lists[e]
        cnt = len(idx_e)
        oe = np.asarray(res.results[e]["out"]).reshape(cap, C)
        out[idx_e] += oe[:cnt]

    return out.reshape(B, T, C), res


def kernel(x, gate_w, w1, w2, w3):
    out, _ = _run(x, gate_w, w1, w2, w3, trace=False)
    return out


# revision 9
# speedup vs baseline: 1.0284x; 1.0284x over previous
# MoE (8 experts, top-2, SwiGLU) Trainium2 kernel.
#
# Strategy (expert-parallel, as suggested by the sharding hint):
#   - Host: compute router (logits -> top-2 -> softmax weights), build the
#     per-expert token lists (the "all-to-all dispatch"), gather + pad the
#     tokens for each expert, and pre-transpose/pad the weight stacks.
#   - Device (8 cores, SPMD, core e owns expert e): dense SwiGLU FFN over the
#     gathered tokens in bf16 with fp32 PSUM accumulation; each token's
#     output is scaled by its routing weight on-device.
#   - Host: scatter-add the two per-expert contributions back to [N, C].
#
# Shapes (hardcoded for this problem):
#   x [2, 2048, 1024] f32, gate_w [8, 1024], w1/w2 [8, 2730, 1024],
#   w3 [8, 1024, 2730].  N = 4096 tokens, C = 1024, H = 2730 (padded 2816).

import numpy as np
import ml_dtypes

NUM_EXPERTS = 8
TOP_K = 2
C = 1024
H = 2730
H2 = 2816  # H padded to a multiple of 128 (zero rows contribute nothing)
KC = C // 128  # 8 contraction chunks over C
MH = H2 // 128  # 22 chunks over padded H
N_CORES = 8

_bf16 = ml_dtypes.bfloat16

_program_cache: dict[int, object] = {}


def _route_host(xt: np.ndarray, gate_w: np.ndarray):
    """Mirror of the reference router in fp32 numpy.

    logits = xt @ gate_w.T; top-2 (ties -> lower index, like jax top_k);
    softmax over the two selected logits.
    """
    logits = xt @ gate_w.T.astype(np.float32)  # [N, E] fp32
    # top-1: first argmax; top-2: argmax with top-1 masked out
    i1 = np.argmax(logits, axis=1)
    n_idx = np.arange(logits.shape[0])
    v1 = logits[n_idx, i1]
    masked = logits.copy()
    masked[n_idx, i1] = -np.inf
    i2 = np.argmax(masked, axis=1)
    v2 = masked[n_idx, i2]
    # softmax over [v1, v2] in fp32 (v1 >= v2)
    e2 = np.exp((v2 - v1).astype(np.float32))
    w1 = (1.0 / (1.0 + e2)).astype(np.float32)
    w2 = (e2 / (1.0 + e2)).astype(np.float32)
    top_idx = np.stack([i1, i2], axis=1)  # [N, 2]
    top_w = np.stack([w1, w2], axis=1)  # [N, 2] fp32
    return top_idx, top_w


def _token_tiles(cap: int):
    tiles = []
    n0 = 0
    while n0 < cap:
        nw = min(512, cap - n0)
        tiles.append((n0, nw))
        n0 += nw
    return tiles


def _build_program(cap: int):
    """Build the SPMD Bass program for per-core token capacity `cap`."""
    import concourse.bass as bass
    import concourse.mybir as mybir
    from concourse import bacc
    from concourse.tile import TileContext

    dt = mybir.dt
    TN = (cap + 127) // 128  # token chunks in phase 2 (last may be ragged)
    tiles = _token_tiles(cap)

    nc = bacc.Bacc(None, target_bir_lowering=False)
    xgT_d = nc.declare_dram_parameter("xgT", [KC, 128, cap], dt.bfloat16, isOutput=False)
    w1T_d = nc.declare_dram_parameter("w1T", [MH, 128, KC, 128], dt.bfloat16, isOutput=False)
    w2T_d = nc.declare_dram_parameter("w2T", [MH, 128, KC, 128], dt.bfloat16, isOutput=False)
    w3T_d = nc.declare_dram_parameter("w3T", [MH, 128, C], dt.bfloat16, isOutput=False)
    wtT_d = nc.declare_dram_parameter("wtT", [128, TN], dt.float32, isOutput=False)
    out_d = nc.declare_dram_parameter("out", [TN, 128, C], dt.float32, isOutput=True)

    with TileContext(nc) as tc:
        with (
            tc.tile_pool(name="big", bufs=1) as big,
            tc.tile_pool(name="wstream", bufs=3) as wpool,
            tc.tile_pool(name="work", bufs=3) as work,
            tc.tile_pool(name="psum", bufs=2, space="PSUM") as psum,
        ):
            # Resident SBUF tensors
            xg_sb = big.tile([128, KC, cap], dt.bfloat16)
            act_sb = big.tile([128, MH, cap], dt.bfloat16)
            w3_sb = big.tile([128, MH, C], dt.bfloat16)
            wt_sb = big.tile([128, TN], dt.float32)

            nc.sync.dma_start(out=wt_sb[:], in_=wtT_d[:])
            for k in range(KC):
                nc.sync.dma_start(out=xg_sb[:, k, :], in_=xgT_d[k])

            # ---- Phase 1: h1/h2 matmuls + SwiGLU -> act_sb ----
            for m in range(MH):
                w1s = wpool.tile([128, KC, 128], dt.bfloat16, tag="w1s")
                w2s = wpool.tile([128, KC, 128], dt.bfloat16, tag="w2s")
                nc.sync.dma_start(out=w1s[:], in_=w1T_d[m])
                nc.sync.dma_start(out=w2s[:], in_=w2T_d[m])
                # w3 chunk for phase 2, loaded here to spread DMA traffic
                nc.sync.dma_start(out=w3_sb[:, m, :], in_=w3T_d[m])

                for (n0, nw) in tiles:
                    ps1 = psum.tile([128, nw], dt.float32, tag="ps1", padded_shape=[128, 512])
                    ps2 = psum.tile([128, nw], dt.float32, tag="ps2", padded_shape=[128, 512])
                    for k in range(KC):
                        nc.tensor.matmul(
                            ps1[:], lhsT=w1s[:, k, :], rhs=xg_sb[:, k, n0:n0 + nw],
                            start=(k == 0), stop=(k == KC - 1),
                        )
                    for k in range(KC):
                        nc.tensor.matmul(
                            ps2[:], lhsT=w2s[:, k, :], rhs=xg_sb[:, k, n0:n0 + nw],
                            start=(k == 0), stop=(k == KC - 1),
                        )
                    tmp = work.tile([128, nw], dt.float32, tag="tmp", padded_shape=[128, 512])
                    nc.scalar.activation(tmp[:], ps1[:], mybir.ActivationFunctionType.Silu)
                    nc.vector.tensor_mul(act_sb[:, m, n0:n0 + nw], tmp[:], ps2[:])

            # ---- Phase 2: out = (act @ w3T) * wt ----
            for tn in range(TN):
                ntok = min(128, cap - tn * 128)  # ragged last chunk
                ps3 = [
                    psum.tile([128, 512], dt.float32, tag=f"ps3_{co}",
                              name=f"ps3_{co}_{tn}")
                    for co in range(2)
                ]
                for m in range(MH):
                    for co in range(2):
                        nc.tensor.matmul(
                            ps3[co][:ntok, :],
                            lhsT=act_sb[:, m, tn * 128:tn * 128 + ntok],
                            rhs=w3_sb[:, m, co * 512:(co + 1) * 512],
                            start=(m == 0), stop=(m == MH - 1),
                        )
                for co in range(2):
                    o_sb = work.tile([128, 512], dt.float32, tag="osb", bufs=4)
                    nc.vector.tensor_scalar_mul(
                        o_sb[:ntok, :], ps3[co][:ntok, :], wt_sb[:ntok, tn:tn + 1]
                    )
                    nc.sync.dma_start(
                        out=out_d[tn][:ntok, co * 512:(co + 1) * 512],
                        in_=o_sb[:ntok, :],
                    )

    nc.finalize()  # runs bacc legalization (e.g. multi-wait split for TRN2)
    return nc


def _prepare_core_inputs(xt, w1, w2, w3, top_idx, top_w):
    """Host-side dispatch: gather tokens per expert, pad, transpose, cast."""
    N = xt.shape[0]
    idx_lists = []
    wt_lists = []
    for e in range(NUM_EXPERTS):
        m0 = top_idx[:, 0] == e
        m1 = top_idx[:, 1] == e
        sel = m0 | m1
        idx_e = np.nonzero(sel)[0]
        wt_e = np.where(m0[idx_e], top_w[idx_e, 0], top_w[idx_e, 1]).astype(np.float32)
        idx_lists.append(idx_e)
        wt_lists.append(wt_e)

    max_cnt = max(len(i) for i in idx_lists)
    cap = max(128, ((max_cnt + 63) // 64) * 64)
    TN = (cap + 127) // 128

    in_maps = []
    for e in range(NUM_EXPERTS):
        idx_e = idx_lists[e]
        cnt = len(idx_e)
        xg = np.zeros((cap, C), np.float32)
        xg[:cnt] = xt[idx_e]
        xgT = np.ascontiguousarray(xg.T.reshape(KC, 128, cap)).astype(_bf16)

        w1p = np.zeros((H2, C), np.float32)
        w1p[:H] = w1[e]
        w2p = np.zeros((H2, C), np.float32)
        w2p[:H] = w2[e]
        w3p = np.zeros((C, H2), np.float32)
        w3p[:, :H] = w3[e]

        # [MH, 128(part=c within chunk), KC, 128(h within chunk)]
        w1T = np.ascontiguousarray(
            w1p.T.reshape(KC, 128, MH, 128).transpose(2, 1, 0, 3)
        ).astype(_bf16)
        w2T = np.ascontiguousarray(
            w2p.T.reshape(KC, 128, MH, 128).transpose(2, 1, 0, 3)
        ).astype(_bf16)
        # [MH, 128(part=h within chunk), C]
        w3T = np.ascontiguousarray(w3p.T.reshape(MH, 128, C)).astype(_bf16)

        wt_pad = np.zeros(TN * 128, np.float32)
        wt_pad[:cnt] = wt_lists[e]
        wtT = np.ascontiguousarray(wt_pad.reshape(TN, 128).T)

        in_maps.append({
            "xgT": xgT, "w1T": w1T, "w2T": w2T, "w3T": w3T, "wtT": wtT,
        })
    return in_maps, idx_lists, cap


def _run(x, gate_w, w1, w2, w3, trace=False):
    from concourse.bass_utils import run_bass_kernel_spmd

    x = np.asarray(x, dtype=np.float32)
    gate_w = np.asarray(gate_w, dtype=np.float32)
    w1 = np.asarray(w1, dtype=np.float32)
    w2 = np.asarray(w2, dtype=np.float32)
    w3 = np.asarray(w3, dtype=np.float32)

    B, T, Cx = x.shape
    assert Cx == C
    xt = x.reshape(-1, C)
    N = xt.shape[0]

    top_idx, top_w = _route_host(xt, gate_w)
    in_maps, idx_lists, cap = _prepare_core_inputs(xt, w1, w2, w3, top_idx, top_w)

    if cap not in _program_cache:
        _program_cache[cap] = _build_program(cap)
    nc = _program_cache[cap]

    res = run_bass_kernel_spmd(nc, in_maps, list(range(N_CORES)), trace=trace)

    out = np.zeros((N, C), np.float32)
    for e in range(NUM_EXPERTS):
        idx_e = idx_lists[e]
        cnt = len(idx_e)
        oe = np.asarray(res.results[e]["out"]).reshape(-1, C)
        out[idx_e] += oe[:cnt]

    return out.reshape(B, T, C), res


def kernel(x, gate_w, w1, w2, w3):
    out, _ = _run(x, gate_w, w1, w2, w3, trace=False)
    return out
